# revision 7
# baseline (speedup 1.0000x reference)
"""Trainium2 Bass kernel for nn_Coembedding (dual-MLP cosine-similarity retrieval).

Computation (see reference):
    mp = relu(molecule @ Wm1.T + bm1) @ Wm2.T + bm2          [N, D]
    pp = relu(protein  @ Wp1.T + bp1) @ Wp2.T + bp2          [M, D]
    out = (pp/|pp| @ (mp/|mp|).T) / temperature              [M, N]

Distribution over 8 NeuronCores:
  - molecule rows (N) sharded 8x for the molecule MLP; normalized embeddings
    (feature-major [D, N/8] bf16) AllGathered so every core holds all N.
  - protein rows (M) sharded 8x; each core computes its own protein MLP shard
    and the [M/8, N] similarity tile.

Perf notes (vs the f32r baseline at 305us):
  - All inputs/weights/activations/outputs are bf16 (fp32 PSUM accumulation):
    halves HBM load traffic (23.7 -> 11.9 MB), which was the gating resource
    for the first 75us (load phase runs at the ~330GB/s HBM ceiling).
  - Queue split: gpsimd=consts+molT+sends+collective+S-out, sync=wm1/wm2+mn
    tiles, scalar=protT, vector=wp1/wp2, so the molecule path (which gates
    the AllGather) is never queued behind protein bytes.
  - 14 warm-up matmuls on a zeroed tile at t=0 release the PE HAM clock
    throttle (1.2 -> 2.4GHz nominal) before real work arrives.
  - Norm chains avoid single-lane DVE reciprocal on the critical path: the
    protein inverse-norm is computed in column form [128, DC] (128-lane
    reciprocal) after a f32r outer-product transpose; the molecule-side
    [1,512] reciprocal overlaps protein-L1 matmuls.
  - Protein L2 runs nh-outer so the norm reduction of each 512-col chunk
    overlaps the next chunk's matmuls.
"""

import numpy as np
import ml_dtypes
from contextlib import ExitStack

import concourse.bass as bass
import concourse.tile as tile
from concourse import bacc, mybir
from concourse.bass_utils import run_bass_kernel_spmd

F32 = mybir.dt.float32
F32R = mybir.dt.float32r
BF16 = mybir.dt.bfloat16
AF = mybir.ActivationFunctionType

N_CORES = 8
N, M, MOL, PROT, D = 4096, 8192, 768, 1280, 1024
NS = N // N_CORES            # 512 molecule rows per core
MS = M // N_CORES            # 1024 protein rows per core
KM, KP, KD = MOL // 128, PROT // 128, D // 128   # 6, 10, 8 contraction chunks
DC = D // 128                # 8 output-feature chunks
EPS = 1e-8

_CACHE: dict = {}


def _build():
    if "nc" in _CACHE:
        return _CACHE["nc"]

    nc = bacc.Bacc("TRN2", target_bir_lowering=False, debug=False,
                   num_devices=N_CORES)

    molT = nc.dram_tensor("molT", [128, KM, NS], BF16, kind="ExternalInput").ap()
    protT = nc.dram_tensor("protT", [128, KP, MS], BF16, kind="ExternalInput").ap()
    wm1 = nc.dram_tensor("wm1", [DC // 2, 128, 2 * KM * 128], BF16, kind="ExternalInput").ap()
    wm2 = nc.dram_tensor("wm2", [DC // 2, 128, 2 * KD * 128], BF16, kind="ExternalInput").ap()
    wp1 = nc.dram_tensor("wp1", [DC // 2, 128, 2 * KP * 128], BF16, kind="ExternalInput").ap()
    wp2 = nc.dram_tensor("wp2", [DC // 2, 128, 2 * KD * 128], BF16, kind="ExternalInput").ap()
    bm1 = nc.dram_tensor("bm1", [128, DC], F32, kind="ExternalInput").ap()
    bm2 = nc.dram_tensor("bm2", [128, DC], F32, kind="ExternalInput").ap()
    bp1 = nc.dram_tensor("bp1", [128, DC], F32, kind="ExternalInput").ap()
    bp2 = nc.dram_tensor("bp2", [128, DC], F32, kind="ExternalInput").ap()
    invtemp = nc.dram_tensor("invtemp", [1, 1], F32, kind="ExternalInput").ap()
    ones_d = nc.dram_tensor("ones", [128, 128], F32, kind="ExternalInput").ap()
    S = nc.dram_tensor("S", [N_CORES, DC, 128, NS], BF16, kind="ExternalOutput").ap()
    S_self = nc.dram_tensor("S_self", [DC, 128, NS], BF16, kind="ExternalOutput").ap()

    with tile.TileContext(nc) as tc, ExitStack() as ctx, \
            nc.allow_low_precision(reason="bf16 data, fp32 accumulation"):
        dram = ctx.enter_context(tc.tile_pool(name="dram", bufs=1, space="DRAM"))
        send = dram.tile([128, DC, NS], BF16)
        recv = dram.tile([N_CORES, 128, DC, NS], BF16, addr_space="Shared")

        sb = ctx.enter_context(tc.tile_pool(name="sb", bufs=1))
        wpool = ctx.enter_context(tc.tile_pool(name="w", bufs=1))
        mn_pool = ctx.enter_context(tc.tile_pool(name="mn", bufs=2))
        st_pool = ctx.enter_context(tc.tile_pool(name="st", bufs=4))
        ps = ctx.enter_context(tc.tile_pool(name="ps", bufs=4, space="PSUM"))
        psn = ctx.enter_context(tc.tile_pool(name="psn", bufs=2, space="PSUM"))
        psb = ctx.enter_context(tc.tile_pool(name="psb", bufs=1, space="PSUM"))

        # ---- PE warm-up: release the HAM clock throttle while DMAs land ----
        warm = sb.tile([128, 512], BF16, tag="warm")
        nc.vector.memset(warm[:], 0.0)
        for _ in range(60):
            wps = ps.tile([128, 512], F32, tag="mm")
            nc.tensor.matmul(wps[:], warm[:, 0:128], warm[:],
                             start=True, stop=True)

        # ---- constants + molecule input on gpsimd queue ----
        ones_col = sb.tile([128, 1], F32R, tag="ones_col")
        nc.gpsimd.dma_start(out=ones_col[:], in_=ones_d[:, 0:1])
        ones_row = sb.tile([1, 128], F32R, tag="ones_row")
        nc.gpsimd.dma_start(out=ones_row[:], in_=ones_d[0:1, :])
        one1 = sb.tile([1, 1], F32, tag="one1")
        nc.gpsimd.dma_start(out=one1[:], in_=ones_d[0:1, 0:1])
        invt = sb.tile([128, 1], F32, tag="invt")
        nc.gpsimd.dma_start(out=invt[:], in_=invtemp.to_broadcast([128, 1]))

        def load_bias(name, ap):
            t = sb.tile([128, DC], F32, tag=name)
            nc.gpsimd.dma_start(out=t[:], in_=ap[:])
            return t

        bm1_s, bm2_s = load_bias("bm1", bm1), load_bias("bm2", bm2)
        bp1_s, bp2_s = load_bias("bp1", bp1), load_bias("bp2", bp2)

        molT_s = sb.tile([128, KM // 2, NS], BF16, tag="molT")
        nc.gpsimd.dma_start(out=molT_s[:], in_=molT[:, 0:KM // 2, :])

        # ---- weights: per-h slabs, all resident; queues chosen so the
        # molecule path is never behind protein bytes ----
        def load_wg(w_dram, kchunks, tag, engine, g):
            t = wpool.tile([128, 2, kchunks, 128], BF16, tag=f"{tag}{g}")
            engine.dma_start(
                out=t[:],
                in_=w_dram[g].rearrange("p (h k m) -> p h k m", h=2, k=kchunks))
            return [t[:, 0], t[:, 1]]

        # consumption-ordered, per-queue sequential DMAs (see module docstring)
        wm1_t = []
        for g in range(DC // 2):
            wm1_t += load_wg(wm1, KM, "wm1_", nc.sync, g)
        molT_b = sb.tile([128, KM - KM // 2, NS], BF16, tag="molTb")
        nc.scalar.dma_start(out=molT_b[:], in_=molT[:, KM // 2:, :])
        wm2_t = []
        for g in range(DC // 2):
            wm2_t += load_wg(wm2, KD, "wm2_", nc.sync, g)
        protT_s = sb.tile([128, KP, MS], BF16, tag="protT")
        nc.scalar.dma_start(out=protT_s[:], in_=protT[:])
        wp1_t = []
        for g in range(DC // 2):
            wp1_t += load_wg(wp1, KP, "wp1_", nc.gpsimd, g)
        wp2_t = []
        for g in range(DC // 2):
            wp2_t += load_wg(wp2, KD, "wp2_", nc.scalar, g)

        def mlp_h(slabs, kchunks, x_tile, lo, width, bias_tile, relu, out_tile, h):
            """out[:, h, lo:lo+width] = act(w[h].T @ x[:, :, lo:lo+width] + b)"""
            pt = ps.tile([128, 512], F32, tag="mm")
            for k in range(kchunks):
                nc.tensor.matmul(
                    pt[:], slabs[h][:, k, :], x_tile[:, k, lo:lo + width],
                    start=(k == 0), stop=(k == kchunks - 1))
            nc.scalar.activation(
                out_tile[:, h, lo:lo + width], pt[:],
                AF.Relu if relu else AF.Identity,
                bias=bias_tile[:, h:h + 1])

        # ================= molecule MLP (N shard) =================
        Hm = sb.tile([128, KD, NS], BF16, tag="hidm")
        for h in range(DC):
            pt = ps.tile([128, 512], F32, tag="mm")
            for k in range(KM):
                x = molT_s[:, k, :] if k < KM // 2 else molT_b[:, k - KM // 2, :]
                nc.tensor.matmul(pt[:], wm1_t[h][:, k, :], x,
                                 start=(k == 0), stop=(k == KM - 1))
            nc.scalar.activation(Hm[:, h, :], pt[:], AF.Relu,
                                 bias=bm1_s[:, h:h + 1])
        # L2 in weight-arrival order, |mp|^2 reduction interleaved (1-chunk lag)
        Mp = sb.tile([128, DC, NS], BF16, tag="embm")
        pn_m = psn.tile([1, NS], F32, tag="psn")
        ORD = list(range(DC))

        def mol_sq(k, first, last):
            sq = st_pool.tile([128, NS], F32R, tag="sq", bufs=4)
            nc.vector.tensor_mul(sq[:], Mp[:, k, :], Mp[:, k, :])
            nc.tensor.matmul(pn_m[:], ones_col[:], sq[:],
                             start=first, stop=last)

        for i, h in enumerate(ORD):
            mlp_h(wm2_t, KD, Hm, 0, NS, bm2_s, False, Mp, h)
            if i >= 1:
                mol_sq(ORD[i - 1], i == 1, False)
        mol_sq(ORD[-1], False, True)
        nsq_m = sb.tile([1, NS], F32R, tag="nsq_m")
        nc.scalar.activation(nsq_m[:], pn_m[:], AF.Sqrt)
        nc.vector.tensor_scalar_max(nsq_m[:], nsq_m[:], EPS)
        inv_m = sb.tile([1, NS], F32R, tag="inv_m")
        nc.vector.reciprocal(inv_m[:], nsq_m[:])   # ~3.3us single-lane, off PE path

        # protein L1 h=0..1: PE filler while the DVE reciprocal runs
        Hp = sb.tile([128, KD, MS], BF16, tag="hidp")
        for h in range(2):
            for nh in range(MS // 512):
                mlp_h(wp1_t, KP, protT_s, nh * 512, 512, bp1_s, True, Hp, h)

        # broadcast inverse norm to [128, NS] and emit normalized bf16 + sends
        pb = psb.tile([128, NS], F32, tag="psb")
        nc.tensor.matmul(pb[:], ones_row[:], inv_m[:], start=True, stop=True)
        binv = sb.tile([128, NS], F32, tag="binv")
        nc.scalar.activation(binv[:], pb[:], AF.Copy)
        Mnb = sb.tile([128, DC, NS], BF16, tag="mnb")
        for k in range(DC):
            nc.vector.tensor_mul(Mnb[:, k, :], Mp[:, k, :], binv[:])
        nc.gpsimd.dma_start(out=send[:], in_=Mnb[:])

        # ================= AllGather molecule embeddings =================
        nc.gpsimd.collective_compute(
            "AllGather",
            mybir.AluOpType.bypass,
            replica_groups=[list(range(N_CORES))],
            ins=[send[:]],
            outs=[recv[:]],
        )

        # ================= protein MLP (M shard), rest =================
        for h in range(2, DC):
            for nh in range(MS // 512):
                mlp_h(wp1_t, KP, protT_s, nh * 512, 512, bp1_s, True, Hp, h)

        # L2 nh-outer so each 512-col chunk's norm reduction overlaps the next
        Pp = sb.tile([128, DC, MS], BF16, tag="embp")
        nrow_p = sb.tile([1, MS], F32, tag="nrow_p")
        for nh in range(MS // 512):
            pn_p = psn.tile([1, 512], F32, tag="psn")

            def prot_sq(k, first, last, nh=nh, pn_p=pn_p):
                sq = st_pool.tile([128, 512], F32R, tag="sq", bufs=4)
                nc.vector.tensor_mul(sq[:], Pp[:, k, nh * 512:(nh + 1) * 512],
                                     Pp[:, k, nh * 512:(nh + 1) * 512])
                nc.tensor.matmul(pn_p[:], ones_col[:], sq[:],
                                 start=first, stop=last)

            for h in range(DC):
                mlp_h(wp2_t, KD, Hp, nh * 512, 512, bp2_s, False, Pp, h)
                if h >= 1:
                    prot_sq(h - 1, h == 1, False)
            prot_sq(DC - 1, False, True)
            nc.scalar.activation(nrow_p[:, nh * 512:(nh + 1) * 512], pn_p[:],
                                 AF.Sqrt)

        # ================= similarity tiles =================
        # protein norms row [1, MS] -> column form [128, DC] via f32r
        # outer-products, then 128-lane max/reciprocal.
        scale_col = sb.tile([128, DC], F32, tag="scale_col")
        pcol = psb.tile([128, DC], F32, tag="psb2")
        for j in range(DC):
            nc.tensor.matmul(
                pcol[:, j:j + 1],
                nrow_p[0:1, j * 128:(j + 1) * 128],
                one1[0:1, 0:1],
                start=(j == 0), stop=(j == DC - 1))
        ncol = sb.tile([128, DC], F32, tag="ncol")
        nc.scalar.activation(ncol[:], pcol[:], AF.Copy)
        nc.vector.tensor_scalar_max(ncol[:], ncol[:], EPS)
        nc.vector.reciprocal(scale_col[:], ncol[:])
        nc.vector.tensor_scalar_mul(scale_col[:], scale_col[:], invt[:, 0:1])

        def sim_tile(mnb_tile, mi, out_ap, qeng):
            pt = ps.tile([128, NS], F32, tag="mm")
            for k in range(KD):
                nc.tensor.matmul(
                    pt[:], Pp[:, k, mi * 128:(mi + 1) * 128],
                    mnb_tile[:, k, :],
                    start=(k == 0), stop=(k == KD - 1),
                )
            stile = st_pool.tile([128, NS], BF16, tag="stile", bufs=4)
            nc.scalar.activation(stile[:], pt[:], AF.Copy,
                                 scale=scale_col[:, mi:mi + 1])
            qeng.dma_start(out=out_ap, in_=stile[:])

        # self tile from the local normalized embeddings: runs during the
        # AllGather wait, keeping the PE busy (and the HAM clock warm)
        for mi in range(MS // 128):
            sim_tile(Mnb, mi, S_self[mi], nc.sync if mi % 2 else nc.gpsimd)

        for c in range(N_CORES):
            mnb = mn_pool.tile([128, DC, NS], BF16, tag="mn")
            nc.sync.dma_start(out=mnb[:], in_=recv[c])
            for mi in range(MS // 128):
                sim_tile(mnb, mi, S[c, mi],
                         nc.sync if (c * 8 + mi) % 2 else nc.gpsimd)

    nc.compile()
    _CACHE["nc"] = nc
    return nc


def _tile_w(W):
    """W [D, K] -> bf16 [DC//2, 128, 2*K]: groups of two output-chunk slabs,
    each group one linear partition-major DMA that completes in issue order."""
    Dout, K = W.shape
    kc = K // 128
    t = W.reshape(DC, 128, kc, 128).transpose(0, 3, 2, 1)   # [h, p, k, m]
    t = t.reshape(DC // 2, 2, 128, kc * 128).transpose(0, 2, 1, 3)
    return np.ascontiguousarray(
        t.reshape(DC // 2, 128, 2 * kc * 128).astype(ml_dtypes.bfloat16))


def _tile_x(Xshard):
    """X [rows, K] -> bf16 [128, KC, rows] feature-major partition-tiled."""
    rows, K = Xshard.shape
    kc = K // 128
    t = Xshard.reshape(rows, kc, 128).transpose(2, 1, 0)    # [p, k, rows]
    return np.ascontiguousarray(t.astype(ml_dtypes.bfloat16))


def kernel(molecule, protein, Wm1, bm1, Wm2, bm2, Wp1, bp1, Wp2, bp2,
           temperature):
    nc = _build()

    molecule = np.asarray(molecule, np.float32)
    protein = np.asarray(protein, np.float32)
    wm1 = _tile_w(np.asarray(Wm1, np.float32))
    wm2 = _tile_w(np.asarray(Wm2, np.float32))
    wp1 = _tile_w(np.asarray(Wp1, np.float32))
    wp2 = _tile_w(np.asarray(Wp2, np.float32))

    def tile_b(b):
        return np.ascontiguousarray(np.asarray(b, np.float32).reshape(DC, 128).T)

    bm1_np, bm2_np = tile_b(bm1), tile_b(bm2)
    bp1_np, bp2_np = tile_b(bp1), tile_b(bp2)
    invt = (1.0 / np.asarray(temperature, np.float32)).reshape(1, 1)
    ones_np = np.ones((128, 128), np.float32)

    in_maps = []
    for c in range(N_CORES):
        in_maps.append({
            "molT": _tile_x(molecule[c * NS:(c + 1) * NS]),
            "protT": _tile_x(protein[c * MS:(c + 1) * MS]),
            "wm1": wm1, "wm2": wm2, "wp1": wp1, "wp2": wp2,
            "bm1": bm1_np, "bm2": bm2_np, "bp1": bp1_np, "bp2": bp2_np,
            "invtemp": invt, "ones": ones_np,
        })

    _CACHE["in_maps"] = in_maps
    res = run_bass_kernel_spmd(nc, in_maps, list(range(N_CORES)))
    out = np.empty((M, N), np.float32)
    for c in range(N_CORES):
        # S block layout [c2, mi, 128, 512] -> rows mi*128+i, cols c2*512+j
        blk = res.results[c]["S"].astype(np.float32)   # [8, 8, 128, 512]
        rows = blk.transpose(1, 2, 0, 3).reshape(MS, N)
        sblk = res.results[c]["S_self"].astype(np.float32)  # [8, 128, 512]
        rows[:, c * NS:(c + 1) * NS] = sblk.reshape(MS, NS)
        out[c * MS:(c + 1) * MS] = rows
    return out


# revision 9
# speedup vs baseline: 1.0260x; 1.0260x over previous
"""Trainium2 Bass kernel for nn_Coembedding (dual-MLP cosine-similarity retrieval).

Computation (see reference):
    mp = relu(molecule @ Wm1.T + bm1) @ Wm2.T + bm2          [N, D]
    pp = relu(protein  @ Wp1.T + bp1) @ Wp2.T + bp2          [M, D]
    out = (pp/|pp| @ (mp/|mp|).T) / temperature              [M, N]

Distribution over 8 NeuronCores:
  - molecule rows (N) sharded 8x for the molecule MLP; normalized embeddings
    (feature-major [D, N/8] bf16) AllGathered so every core holds all N.
  - protein rows (M) sharded 8x; each core computes its own protein MLP shard
    and the [M/8, N] similarity tile.

Perf notes (vs the f32r baseline at 305us):
  - All inputs/weights/activations/outputs are bf16 (fp32 PSUM accumulation):
    halves HBM load traffic (23.7 -> 11.9 MB), which was the gating resource
    for the first 75us (load phase runs at the ~330GB/s HBM ceiling).
  - Queue split: gpsimd=consts+molT+sends+collective+S-out, sync=wm1/wm2+mn
    tiles, scalar=protT, vector=wp1/wp2, so the molecule path (which gates
    the AllGather) is never queued behind protein bytes.
  - 14 warm-up matmuls on a zeroed tile at t=0 release the PE HAM clock
    throttle (1.2 -> 2.4GHz nominal) before real work arrives.
  - Norm chains avoid single-lane DVE reciprocal on the critical path: the
    protein inverse-norm is computed in column form [128, DC] (128-lane
    reciprocal) after a f32r outer-product transpose; the molecule-side
    [1,512] reciprocal overlaps protein-L1 matmuls.
  - Protein L2 runs nh-outer so the norm reduction of each 512-col chunk
    overlaps the next chunk's matmuls.
"""

import numpy as np
import ml_dtypes
from contextlib import ExitStack

import concourse.bass as bass
import concourse.tile as tile
from concourse import bacc, mybir
from concourse.bass_utils import run_bass_kernel_spmd

F32 = mybir.dt.float32
F32R = mybir.dt.float32r
BF16 = mybir.dt.bfloat16
AF = mybir.ActivationFunctionType

N_CORES = 8
N, M, MOL, PROT, D = 4096, 8192, 768, 1280, 1024
NS = N // N_CORES            # 512 molecule rows per core
MS = M // N_CORES            # 1024 protein rows per core
KM, KP, KD = MOL // 128, PROT // 128, D // 128   # 6, 10, 8 contraction chunks
DC = D // 128                # 8 output-feature chunks
EPS = 1e-8

_CACHE: dict = {}


def _build():
    if "nc" in _CACHE:
        return _CACHE["nc"]

    nc = bacc.Bacc("TRN2", target_bir_lowering=False, debug=False,
                   num_devices=N_CORES)

    molT = nc.dram_tensor("molT", [128, KM, NS], BF16, kind="ExternalInput").ap()
    protT = nc.dram_tensor("protT", [128, KP, MS], BF16, kind="ExternalInput").ap()
    wm1 = nc.dram_tensor("wm1", [DC // 2, 128, 2 * KM * 128], BF16, kind="ExternalInput").ap()
    wm2 = nc.dram_tensor("wm2", [DC // 2, 128, 2 * KD * 128], BF16, kind="ExternalInput").ap()
    wp1 = nc.dram_tensor("wp1", [DC // 2, 128, 2 * KP * 128], BF16, kind="ExternalInput").ap()
    wp2 = nc.dram_tensor("wp2", [DC // 2, 128, 2 * KD * 128], BF16, kind="ExternalInput").ap()
    bm1 = nc.dram_tensor("bm1", [128, DC], F32, kind="ExternalInput").ap()
    bm2 = nc.dram_tensor("bm2", [128, DC], F32, kind="ExternalInput").ap()
    bp1 = nc.dram_tensor("bp1", [128, DC], F32, kind="ExternalInput").ap()
    bp2 = nc.dram_tensor("bp2", [128, DC], F32, kind="ExternalInput").ap()
    invtemp = nc.dram_tensor("invtemp", [1, 1], F32, kind="ExternalInput").ap()
    ones_d = nc.dram_tensor("ones", [128, 128], F32, kind="ExternalInput").ap()
    S = nc.dram_tensor("S", [N_CORES, DC, 128, NS], BF16, kind="ExternalOutput").ap()

    with tile.TileContext(nc) as tc, ExitStack() as ctx, \
            nc.allow_low_precision(reason="bf16 data, fp32 accumulation"):
        dram = ctx.enter_context(tc.tile_pool(name="dram", bufs=1, space="DRAM"))
        sendA = dram.tile([128, DC, NS // 2], BF16)
        sendB = dram.tile([128, DC, NS // 2], BF16)
        recvA = dram.tile([N_CORES, 128, DC, NS // 2], BF16, addr_space="Shared")
        recvB = dram.tile([N_CORES, 128, DC, NS // 2], BF16, addr_space="Shared")

        sb = ctx.enter_context(tc.tile_pool(name="sb", bufs=1))
        wpool = ctx.enter_context(tc.tile_pool(name="w", bufs=1))
        mn_pool = ctx.enter_context(tc.tile_pool(name="mn", bufs=2))
        st_pool = ctx.enter_context(tc.tile_pool(name="st", bufs=4))
        ps = ctx.enter_context(tc.tile_pool(name="ps", bufs=4, space="PSUM"))
        psn = ctx.enter_context(tc.tile_pool(name="psn", bufs=2, space="PSUM"))
        psb = ctx.enter_context(tc.tile_pool(name="psb", bufs=1, space="PSUM"))

        # ---- PE warm-up: release the HAM clock throttle while DMAs land ----
        warm = sb.tile([128, 512], BF16, tag="warm")
        nc.vector.memset(warm[:], 0.0)
        for _ in range(60):
            wps = ps.tile([128, 512], F32, tag="mm")
            nc.tensor.matmul(wps[:], warm[:, 0:128], warm[:],
                             start=True, stop=True)

        # ---- constants + molecule input on gpsimd queue ----
        ones_col = sb.tile([128, 1], F32R, tag="ones_col")
        nc.gpsimd.dma_start(out=ones_col[:], in_=ones_d[:, 0:1])
        ones_row = sb.tile([1, 128], F32R, tag="ones_row")
        nc.gpsimd.dma_start(out=ones_row[:], in_=ones_d[0:1, :])
        one1 = sb.tile([1, 1], F32, tag="one1")
        nc.gpsimd.dma_start(out=one1[:], in_=ones_d[0:1, 0:1])
        invt = sb.tile([128, 1], F32, tag="invt")
        nc.gpsimd.dma_start(out=invt[:], in_=invtemp.to_broadcast([128, 1]))

        def load_bias(name, ap):
            t = sb.tile([128, DC], F32, tag=name)
            nc.gpsimd.dma_start(out=t[:], in_=ap[:])
            return t

        bm1_s, bm2_s = load_bias("bm1", bm1), load_bias("bm2", bm2)
        bp1_s, bp2_s = load_bias("bp1", bp1), load_bias("bp2", bp2)

        molT_s = sb.tile([128, KM // 2, NS], BF16, tag="molT")
        nc.gpsimd.dma_start(out=molT_s[:], in_=molT[:, 0:KM // 2, :])

        # ---- weights: per-h slabs, all resident; queues chosen so the
        # molecule path is never behind protein bytes ----
        def load_wg(w_dram, kchunks, tag, engine, g):
            t = wpool.tile([128, 2, kchunks, 128], BF16, tag=f"{tag}{g}")
            engine.dma_start(
                out=t[:],
                in_=w_dram[g].rearrange("p (h k m) -> p h k m", h=2, k=kchunks))
            return [t[:, 0], t[:, 1]]

        # consumption-ordered, per-queue sequential DMAs (see module docstring)
        wm1_t = []
        for g in range(DC // 2):
            wm1_t += load_wg(wm1, KM, "wm1_", nc.sync, g)
        molT_b = sb.tile([128, KM - KM // 2, NS], BF16, tag="molTb")
        nc.scalar.dma_start(out=molT_b[:], in_=molT[:, KM // 2:, :])
        wm2_t = []
        for g in range(DC // 2):
            wm2_t += load_wg(wm2, KD, "wm2_", nc.sync, g)
        protT_s = sb.tile([128, KP, MS], BF16, tag="protT")
        nc.scalar.dma_start(out=protT_s[:], in_=protT[:])
        wp1_t = []
        for g in range(DC // 2):
            wp1_t += load_wg(wp1, KP, "wp1_", nc.gpsimd, g)
        wp2_t = []
        for g in range(DC // 2):
            wp2_t += load_wg(wp2, KD, "wp2_", nc.scalar, g)

        def mlp_h(slabs, kchunks, x_tile, lo, width, bias_tile, relu, out_tile, h):
            """out[:, h, lo:lo+width] = act(w[h].T @ x[:, :, lo:lo+width] + b)"""
            pt = ps.tile([128, 512], F32, tag="mm")
            for k in range(kchunks):
                nc.tensor.matmul(
                    pt[:], slabs[h][:, k, :], x_tile[:, k, lo:lo + width],
                    start=(k == 0), stop=(k == kchunks - 1))
            nc.scalar.activation(
                out_tile[:, h, lo:lo + width], pt[:],
                AF.Relu if relu else AF.Identity,
                bias=bias_tile[:, h:h + 1])

        # ================= molecule MLP (N shard) =================
        Hm = sb.tile([128, KD, NS], BF16, tag="hidm")
        for h in range(DC):
            pt = ps.tile([128, 512], F32, tag="mm")
            for k in range(KM):
                x = molT_s[:, k, :] if k < KM // 2 else molT_b[:, k - KM // 2, :]
                nc.tensor.matmul(pt[:], wm1_t[h][:, k, :], x,
                                 start=(k == 0), stop=(k == KM - 1))
            nc.scalar.activation(Hm[:, h, :], pt[:], AF.Relu,
                                 bias=bm1_s[:, h:h + 1])
        # L2 in weight-arrival order, |mp|^2 reduction interleaved (1-chunk lag)
        Mp = sb.tile([128, DC, NS], BF16, tag="embm")
        pn_m = psn.tile([1, NS], F32, tag="psn")
        ORD = list(range(DC))

        def mol_sq(k, first, last):
            sq = st_pool.tile([128, NS], F32R, tag="sq", bufs=4)
            nc.vector.tensor_mul(sq[:], Mp[:, k, :], Mp[:, k, :])
            nc.tensor.matmul(pn_m[:], ones_col[:], sq[:],
                             start=first, stop=last)

        for i, h in enumerate(ORD):
            mlp_h(wm2_t, KD, Hm, 0, NS, bm2_s, False, Mp, h)
            if i >= 1:
                mol_sq(ORD[i - 1], i == 1, False)
        mol_sq(ORD[-1], False, True)
        nsq_m = sb.tile([1, NS], F32R, tag="nsq_m")
        nc.scalar.activation(nsq_m[:], pn_m[:], AF.Sqrt)
        nc.vector.tensor_scalar_max(nsq_m[:], nsq_m[:], EPS)
        inv_m = sb.tile([1, NS], F32R, tag="inv_m")
        nc.vector.reciprocal(inv_m[:], nsq_m[:])   # ~3.3us single-lane, off PE path

        # protein L1 h=0..1: PE filler while the DVE reciprocal runs
        Hp = sb.tile([128, KD, MS], BF16, tag="hidp")
        for h in range(2):
            for nh in range(MS // 512):
                mlp_h(wp1_t, KP, protT_s, nh * 512, 512, bp1_s, True, Hp, h)

        # broadcast inverse norm to [128, NS] and emit normalized bf16 + sends
        pb = psb.tile([128, NS], F32, tag="psb")
        nc.tensor.matmul(pb[:], ones_row[:], inv_m[:], start=True, stop=True)
        binv = sb.tile([128, NS], F32, tag="binv")
        nc.scalar.activation(binv[:], pb[:], AF.Copy)
        Mnb = sb.tile([128, DC, NS], BF16, tag="mnb")
        for k in range(DC):
            nc.vector.tensor_mul(Mnb[:, k, :], Mp[:, k, :], binv[:])
        nc.gpsimd.dma_start(out=sendA[:], in_=Mnb[:, :, 0:NS // 2])
        nc.gpsimd.dma_start(out=sendB[:], in_=Mnb[:, :, NS // 2:])

        # ===== AllGather molecule embeddings, split in two so similarity on
        # the first half can start ~25us after the CC stream wakes up =====
        nc.gpsimd.collective_compute(
            "AllGather", mybir.AluOpType.bypass,
            replica_groups=[list(range(N_CORES))],
            ins=[sendA[:]], outs=[recvA[:]])
        nc.gpsimd.collective_compute(
            "AllGather", mybir.AluOpType.bypass,
            replica_groups=[list(range(N_CORES))],
            ins=[sendB[:]], outs=[recvB[:]])

        # ================= protein MLP (M shard), rest =================
        for h in range(2, DC):
            for nh in range(MS // 512):
                mlp_h(wp1_t, KP, protT_s, nh * 512, 512, bp1_s, True, Hp, h)

        # L2 nh-outer so each 512-col chunk's norm reduction overlaps the next
        Pp = sb.tile([128, DC, MS], BF16, tag="embp")
        nrow_p = sb.tile([1, MS], F32, tag="nrow_p")
        for nh in range(MS // 512):
            pn_p = psn.tile([1, 512], F32, tag="psn")

            def prot_sq(k, first, last, nh=nh, pn_p=pn_p):
                sq = st_pool.tile([128, 512], F32R, tag="sq", bufs=4)
                nc.vector.tensor_mul(sq[:], Pp[:, k, nh * 512:(nh + 1) * 512],
                                     Pp[:, k, nh * 512:(nh + 1) * 512])
                nc.tensor.matmul(pn_p[:], ones_col[:], sq[:],
                                 start=first, stop=last)

            for h in range(DC):
                mlp_h(wp2_t, KD, Hp, nh * 512, 512, bp2_s, False, Pp, h)
                if h >= 1:
                    prot_sq(h - 1, h == 1, False)
            prot_sq(DC - 1, False, True)
            nc.scalar.activation(nrow_p[:, nh * 512:(nh + 1) * 512], pn_p[:],
                                 AF.Sqrt)

        # ================= similarity tiles =================
        # protein norms row [1, MS] -> column form [128, DC] via f32r
        # outer-products, then 128-lane max/reciprocal.
        scale_col = sb.tile([128, DC], F32, tag="scale_col")
        pcol = psb.tile([128, DC], F32, tag="psb2")
        for j in range(DC):
            nc.tensor.matmul(
                pcol[:, j:j + 1],
                nrow_p[0:1, j * 128:(j + 1) * 128],
                one1[0:1, 0:1],
                start=(j == 0), stop=(j == DC - 1))
        ncol = sb.tile([128, DC], F32, tag="ncol")
        nc.scalar.activation(ncol[:], pcol[:], AF.Copy)
        nc.vector.tensor_scalar_max(ncol[:], ncol[:], EPS)
        nc.vector.reciprocal(scale_col[:], ncol[:])
        nc.vector.tensor_scalar_mul(scale_col[:], scale_col[:], invt[:, 0:1])

        # HAM keepalive: bridge the small gap until AG-A lands
        for _ in range(20):
            wps = ps.tile([128, 512], F32, tag="mm")
            nc.tensor.matmul(wps[:], warm[:, 0:128], warm[:],
                             start=True, stop=True)

        HN = NS // 2
        for half, rc in ((0, recvA), (1, recvB)):
            for c in range(N_CORES):
                mnb = mn_pool.tile([128, DC, HN], BF16, tag="mn")
                nc.sync.dma_start(out=mnb[:], in_=rc[c])
                for mi in range(MS // 128):
                    pt = ps.tile([128, 512], F32, tag="mm")
                    for k in range(KD):
                        nc.tensor.matmul(
                            pt[:, 0:HN], Pp[:, k, mi * 128:(mi + 1) * 128],
                            mnb[:, k, :],
                            start=(k == 0), stop=(k == KD - 1),
                        )
                    stile = st_pool.tile([128, HN], BF16, tag="stile", bufs=4)
                    nc.scalar.activation(stile[:], pt[:, 0:HN], AF.Copy,
                                         scale=scale_col[:, mi:mi + 1])
                    (nc.sync if (c * 8 + mi) % 2 else nc.gpsimd).dma_start(
                        out=S[c, mi, :, half * HN:(half + 1) * HN],
                        in_=stile[:])

    nc.compile()
    _CACHE["nc"] = nc
    return nc


def _tile_w(W):
    """W [D, K] -> bf16 [DC//2, 128, 2*K]: groups of two output-chunk slabs,
    each group one linear partition-major DMA that completes in issue order."""
    Dout, K = W.shape
    kc = K // 128
    t = W.reshape(DC, 128, kc, 128).transpose(0, 3, 2, 1)   # [h, p, k, m]
    t = t.reshape(DC // 2, 2, 128, kc * 128).transpose(0, 2, 1, 3)
    return np.ascontiguousarray(
        t.reshape(DC // 2, 128, 2 * kc * 128).astype(ml_dtypes.bfloat16))


def _tile_x(Xshard):
    """X [rows, K] -> bf16 [128, KC, rows] feature-major partition-tiled."""
    rows, K = Xshard.shape
    kc = K // 128
    t = Xshard.reshape(rows, kc, 128).transpose(2, 1, 0)    # [p, k, rows]
    return np.ascontiguousarray(t.astype(ml_dtypes.bfloat16))


def kernel(molecule, protein, Wm1, bm1, Wm2, bm2, Wp1, bp1, Wp2, bp2,
           temperature):
    nc = _build()

    molecule = np.asarray(molecule, np.float32)
    protein = np.asarray(protein, np.float32)
    wm1 = _tile_w(np.asarray(Wm1, np.float32))
    wm2 = _tile_w(np.asarray(Wm2, np.float32))
    wp1 = _tile_w(np.asarray(Wp1, np.float32))
    wp2 = _tile_w(np.asarray(Wp2, np.float32))

    def tile_b(b):
        return np.ascontiguousarray(np.asarray(b, np.float32).reshape(DC, 128).T)

    bm1_np, bm2_np = tile_b(bm1), tile_b(bm2)
    bp1_np, bp2_np = tile_b(bp1), tile_b(bp2)
    invt = (1.0 / np.asarray(temperature, np.float32)).reshape(1, 1)
    ones_np = np.ones((128, 128), np.float32)

    in_maps = []
    for c in range(N_CORES):
        in_maps.append({
            "molT": _tile_x(molecule[c * NS:(c + 1) * NS]),
            "protT": _tile_x(protein[c * MS:(c + 1) * MS]),
            "wm1": wm1, "wm2": wm2, "wp1": wp1, "wp2": wp2,
            "bm1": bm1_np, "bm2": bm2_np, "bp1": bp1_np, "bp2": bp2_np,
            "invtemp": invt, "ones": ones_np,
        })

    _CACHE["in_maps"] = in_maps
    res = run_bass_kernel_spmd(nc, in_maps, list(range(N_CORES)))
    out = np.empty((M, N), np.float32)
    for c in range(N_CORES):
        # S block layout [c2, mi, 128, 512] -> rows mi*128+i, cols c2*512+j
        blk = res.results[c]["S"].astype(np.float32)   # [8, 8, 128, 512]
        out[c * MS:(c + 1) * MS] = blk.transpose(1, 2, 0, 3).reshape(MS, N)
    return out
